# revision 10
# baseline (speedup 1.0000x reference)
"""Distributed GCN classifier kernel for Trainium2 (8 NeuronCores).

Strategy (sharding_hint): nodes block-partitioned 6250/core; edges
partitioned by dst-owner core, sorted by (dst-window, src-half, dst);
small weights replicated. Halo exchange = AllGather of the scaled node
feature tables (fp16). Segment-sum = dma_gather of source rows + one-hot
indicator (DVE is_equal) + PE matmul accumulation per 128-dst window.

Algebra (exact refactor of reference):
  conv: agg = dinv*(segsum((dinv*h)[src]) ) + dinv^2*h; both convs
  aggregate in the 128-dim space before/after the weight matmul.
  edge head folded: edge_x = y[src]@Qs + y[dst]@Qd + ep@Wep + ce with
  y = h2 - b2, Qs/Qd = Wd@We[:64]/We[64:128], all computed on device.
"""
import os
import numpy as np
import ml_dtypes

import concourse.bacc as bacc
import concourse.bass as bass
import concourse.mybir as mybir
import concourse.tile as tile
from concourse.bass_utils import run_bass_kernel_spmd
from concourse import library_config

dt = mybir.dt

M = 8
N = 50000
NP = N // M
F = 128
H1, H2, HD, PP, C = 256, 128, 64, 3, 2
W = 128                      # dst window
HB = N // 2                  # src bucket boundary (int16 reach)
TILE = 128
CALL_TILES = 32              # gather-call granularity (4096 idxs)

last_run_info = {}


def cdiv(a, b):
    return -(-a // b)


# --------------------------------------------------------------------------
# host-side graph preprocessing
# --------------------------------------------------------------------------

def prep(src, dst, n=N, m=M):
    np_ = n // m
    nw = cdiv(np_, W)
    hb = n // 2
    src = np.asarray(src, np.int64)
    dst = np.asarray(dst, np.int64)
    owner = dst // np_
    dstl = dst - owner * np_
    win = dstl // W
    bucket = (src >= hb).astype(np.int64)
    srcl = src - bucket * hb

    deg = np.bincount(dst, minlength=n).astype(np.float32) + 1.0

    counts = np.zeros((m, nw, 2), np.int64)
    per_core = []
    for c in range(m):
        sel = np.nonzero(owner == c)[0]
        order = np.lexsort((dstl[sel], bucket[sel], win[sel]))
        eids = sel[order]
        per_core.append(eids)
        np.add.at(counts[c], (win[sel], bucket[sel]), 1)

    T = cdiv(counts.max(axis=0), TILE)       # [nw, 2]
    if T.sum() == 0:
        T[0, 0] = 1
    toff = np.zeros((nw, 2), np.int64)
    acc = 0
    for b in range(2):
        for w_ in range(nw):
            toff[w_, b] = acc
            acc += T[w_, b]
    TT = acc
    TT0 = int(T[:, 0].sum())
    L = TT * TILE

    # gather-call plan: fixed chunks within each bucket region, then the
    # two regions' calls interleaved for issue order
    calls = []          # (tile_start, ntiles, bucket)
    r0 = [(s, min(CALL_TILES, TT0 - s), 0) for s in range(0, TT0, CALL_TILES)]
    r1 = [(TT0 + s, min(CALL_TILES, TT - TT0 - s), 1)
          for s in range(0, TT - TT0, CALL_TILES)]
    for i in range(max(len(r0), len(r1))):
        if i < len(r0):
            calls.append(r0[i])
        if i < len(r1):
            calls.append(r1[i])
    # column base (in int16 idx columns) per call, in TILE-STREAM order:
    # the wrapped idx array is laid out call-after-call in issue order
    colbase = {}
    cb = 0
    for (s, nt, b) in calls:
        colbase[s] = cb
        cb += nt * TILE // 16
    GC = cb

    struct = dict(T=T, toff=toff, TT=TT, TT0=TT0, L=L, nw=nw, np_=np_,
                  calls=calls, colbase=colbase, GC=GC, n=n, m=m, hb=hb)

    cores = []
    for c in range(m):
        eids = per_core[c]
        cw = win[eids]
        cb_ = bucket[eids]
        slot = np.full(L, -1, np.int64)
        g16 = np.zeros(L, np.int64)
        dsl = np.full(L, -1.0, np.float32)
        ed16 = np.zeros(L, np.int64)
        for b in range(2):
            msk_b = cb_ == b
            ee_b = eids[msk_b]
            ww_b = cw[msk_b]
            # stable order within (b, w) already sorted by lexsort
            for w_ in range(nw):
                ee = ee_b[ww_b == w_]
                nn = ee.shape[0]
                base = toff[w_, b] * TILE
                slot[base:base + nn] = ee
                g16[base:base + nn] = srcl[ee]
                dsl[base:base + nn] = (dstl[ee] - w_ * W).astype(np.float32)
                ed16[base:base + nn] = dstl[ee]
        cores.append(dict(slot=slot, g16=g16, dsl=dsl, ed16=ed16,
                          deg=deg[c * np_:(c + 1) * np_]))
    return cores, struct


def wrap_idx(flat, struct):
    """flat [L] int -> wrapped [128, GC] int16 in per-call layout."""
    out = np.zeros((16, struct["GC"]), np.int16)
    for (s, nt, b) in struct["calls"]:
        cb = struct["colbase"][s]
        n = nt * TILE
        blk = flat[s * TILE:s * TILE + n].astype(np.int16)
        out[:, cb:cb + n // 16] = blk.reshape(n // 16, 16).T
    return np.tile(out, (8, 1))


# --------------------------------------------------------------------------
# device program
# --------------------------------------------------------------------------

def build_nc(S):
    nw, np_, TT, L, GC = S["nw"], S["np_"], S["TT"], S["L"], S["GC"]
    n, hb = S["n"], S["hb"]
    T, toff, calls, colbase = S["T"], S["toff"], S["calls"], S["colbase"]
    m = S["m"]
    f16 = dt.float16
    f32 = dt.float32

    nc = bacc.Bacc("TRN2", target_bir_lowering=False, debug=False,
                   enable_asserts=False, num_devices=m)

    def inp(name, shape, d=f32):
        return nc.dram_tensor(name, shape, d, kind="ExternalInput").ap()

    x_blk = inp("x_blk", [np_, F])
    deg_t = inp("deg_t", [128, nw])
    gidx = inp("gidx", [128, GC], dt.int16)
    edst = inp("edst", [128, GC], dt.int16)
    dslot = inp("dslot", [128, TT])
    ept = inp("ept", [PP, L], f16)
    iota_r = inp("iota_r", [128, 128], f16)
    ident = inp("ident", [128, 128])
    W1_i = inp("W1_i", [F, H1])
    W2a_i = inp("W2a_i", [128, H2], f16)
    W2b_i = inp("W2b_i", [128, H2], f16)
    WdT_i = inp("WdT_i", [HD, F])
    Wn_i = inp("Wn_i", [HD, C])
    Wes_i = inp("Wes_i", [HD, C])
    Wed_i = inp("Wed_i", [HD, C])
    Wep_i = inp("Wep_i", [PP, C], f16)
    b1c_i = inp("b1c_i", [128, 2])
    b2c_i = inp("b2c_i", [128, 1])
    bdc_i = inp("bdc_i", [HD, 1])
    bnr_i = inp("bnr_i", [1, C])
    ber_i = inp("ber_i", [1, C])
    one_i = inp("one_i", [1, 1])

    nodeT = nc.dram_tensor("nodeT", [C, np_], f32, kind="ExternalOutput").ap()
    eoutT = nc.dram_tensor("eoutT", [C, L], f32, kind="ExternalOutput").ap()
    dbg = nc.dram_tensor("dbg", [128, F], f32, kind="ExternalOutput").ap()

    u_blk = nc.dram_tensor("u_blk", [np_, F], f16)
    u_full = nc.dram_tensor("u_full", [n, F], f16, addr_space="Shared")
    w_blk = nc.dram_tensor("w_blk", [np_, F], f16)
    w_full = nc.dram_tensor("w_full", [n, F], f16, addr_space="Shared")
    y_blk = nc.dram_tensor("y_blk", [np_, F], f16)
    y_full = nc.dram_tensor("y_full", [n, F], f16, addr_space="Shared")

    rg = [list(range(m))]
    ID = mybir.ActivationFunctionType.Identity
    RELU = mybir.ActivationFunctionType.Relu
    SQRT = mybir.ActivationFunctionType.Sqrt
    EQ = mybir.AluOpType.is_equal
    MUL = mybir.AluOpType.mult
    ADD = mybir.AluOpType.add

    PH = int(os.environ.get("GNN_PHASE", "9"))
    SUB = int(os.environ.get("GNN_SUB", "9"))
    SP = False
    with tile.TileContext(nc) as tc:
        with (
            tc.tile_pool(name="const", bufs=1) as cp,
            tc.tile_pool(name="big", bufs=1) as bigp,
            tc.tile_pool(name="nodebuf", bufs=1) as nbp,
            tc.tile_pool(name="sb", bufs=3) as sb,
            tc.tile_pool(name="msg0", bufs=2) as mp0,
            tc.tile_pool(name="msg1", bufs=2) as mp1,
            tc.tile_pool(name="ind", bufs=4) as indp,
            tc.tile_pool(name="gix", bufs=4) as gixp,
            tc.tile_pool(name="edge", bufs=2) as edp,
            tc.tile_pool(name="psW", bufs=2, space="PSUM") as psW,
            tc.tile_pool(name="psT", bufs=2, space="PSUM") as psT,
            tc.tile_pool(name="psH", bufs=2, space="PSUM") as psH,
            tc.tile_pool(name="psZ", bufs=2, space="PSUM") as psZ,
        ):
            nc.gpsimd.load_library(library_config.mlp)

            # ---------- constants ----------
            iota_sb = cp.tile([128, 128], f16)
            nc.sync.dma_start(iota_sb[:], iota_r)
            ident_sb = cp.tile([128, 128], f32)
            nc.sync.dma_start(ident_sb[:], ident)
            dslot_sb = cp.tile([128, TT], f32)
            nc.sync.dma_start(dslot_sb[:], dslot)
            W1_sb = cp.tile([F, H1], f32)
            nc.sync.dma_start(W1_sb[:], W1_i)
            W2a_sb = cp.tile([128, H2], f16)
            nc.sync.dma_start(W2a_sb[:], W2a_i)
            W2b_sb = cp.tile([128, H2], f16)
            nc.sync.dma_start(W2b_sb[:], W2b_i)
            WdT_sb = cp.tile([HD, F], f32)
            nc.sync.dma_start(WdT_sb[:], WdT_i)
            Wn_sb = cp.tile([HD, C], f32)
            nc.sync.dma_start(Wn_sb[:], Wn_i)
            Wes_sb = cp.tile([HD, C], f32)
            nc.sync.dma_start(Wes_sb[:], Wes_i)
            Wed_sb = cp.tile([HD, C], f32)
            nc.sync.dma_start(Wed_sb[:], Wed_i)
            Wep_sb = cp.tile([PP, C], f16)
            nc.sync.dma_start(Wep_sb[:], Wep_i)
            b1c_sb = cp.tile([128, 2], f32)
            nc.sync.dma_start(b1c_sb[:], b1c_i)
            b2c_sb = cp.tile([128, 1], f32)
            nc.sync.dma_start(b2c_sb[:], b2c_i)
            bdc_sb = cp.tile([HD, 1], f32)
            nc.sync.dma_start(bdc_sb[:], bdc_i)
            bnr_sb = cp.tile([1, C], f32)
            nc.sync.dma_start(bnr_sb[:], bnr_i)
            ber_sb = cp.tile([1, C], f32)
            nc.sync.dma_start(ber_sb[:], ber_i)
            one_sb = cp.tile([1, 1], f32)
            nc.sync.dma_start(one_sb[:], one_i)

            # dinv = sqrt(1/deg)
            deg_sb = cp.tile([128, nw], f32)
            nc.sync.dma_start(deg_sb[:], deg_t)
            rc_sb = cp.tile([128, nw], f32)
            nc.vector.reciprocal(rc_sb[:], deg_sb[:])
            dinv = cp.tile([128, nw], f32)
            nc.scalar.activation(dinv[:], rc_sb[:], SQRT)

            # ---------- weight folding ----------
            def fold_q(rhs_ap, name):
                pq = psH.tile([128, C], f32, tag="ph")
                nc.tensor.matmul(pq[:], lhsT=WdT_sb[:], rhs=rhs_ap,
                                 start=True, stop=True)
                q32 = cp.tile([128, C], f32, tag=name + "32")
                nc.vector.tensor_copy(q32[:], pq[:])
                q16 = cp.tile([128, C], f16, tag=name + "16")
                nc.vector.tensor_copy(q16[:], pq[:])
                return q32, q16

            Qn_sb, _ = fold_q(Wn_sb[:], "qn")
            Qs_sb, Qs16 = fold_q(Wes_sb[:], "qs")
            Qd_sb, Qd16 = fold_q(Wed_sb[:], "qd")

            pc_ = psZ.tile([C, 1], f32, tag="pz")
            nc.tensor.matmul(pc_[:], lhsT=Qn_sb[:], rhs=b2c_sb[:],
                             start=True, stop=False)
            nc.tensor.matmul(pc_[:], lhsT=Wn_sb[:], rhs=bdc_sb[:],
                             start=False, stop=False)
            nc.tensor.matmul(pc_[:], lhsT=bnr_sb[:], rhs=one_sb[:],
                             start=False, stop=True)
            cn_sb = cp.tile([C, 1], f32)
            nc.vector.tensor_copy(cn_sb[:], pc_[:])

            pe_ = psZ.tile([C, 1], f32, tag="pz")
            nc.tensor.matmul(pe_[:], lhsT=Qs_sb[:], rhs=b2c_sb[:],
                             start=True, stop=False)
            nc.tensor.matmul(pe_[:], lhsT=Qd_sb[:], rhs=b2c_sb[:],
                             start=False, stop=False)
            nc.tensor.matmul(pe_[:], lhsT=Wes_sb[:], rhs=bdc_sb[:],
                             start=False, stop=False)
            nc.tensor.matmul(pe_[:], lhsT=Wed_sb[:], rhs=bdc_sb[:],
                             start=False, stop=False)
            nc.tensor.matmul(pe_[:], lhsT=ber_sb[:], rhs=one_sb[:],
                             start=False, stop=True)
            ce_sb = cp.tile([C, 1], f32)
            nc.vector.tensor_copy(ce_sb[:], pe_[:])

            # ---------- phase A: u = dinv*x ----------
            u2_all = nbp.tile([128, nw * F], f32, tag="nodebuf")
            for t in range(nw):
                pcnt = min(128, np_ - t * 128)
                xt = sb.tile([128, F], f32, tag="xt")
                nc.sync.dma_start(xt[:pcnt, :], x_blk[t * 128:t * 128 + pcnt, :])
                ut = sb.tile([128, F], f32, tag="ut")
                nc.vector.tensor_scalar_mul(ut[:], xt[:], dinv[:, t:t + 1])
                uh = sb.tile([128, F], f16, tag="uh")
                nc.vector.tensor_copy(uh[:], ut[:])
                nc.sync.dma_start(u_blk[t * 128:t * 128 + pcnt, :], uh[:pcnt, :])
                nc.vector.tensor_scalar_mul(
                    u2_all[:, t * F:(t + 1) * F], ut[:], dinv[:, t:t + 1])
            nc.gpsimd.collective_compute(
                "AllGather", mybir.AluOpType.bypass, replica_groups=rg,
                ins=[u_blk[:]], outs=[u_full[:]])


            # ---------- shared segsum machinery ----------
            def segsum(table_full, flush):
                msg_tiles = {}
                for (s, nt, b) in calls:
                    pool = mp0 if b == 0 else mp1
                    mt = pool.tile([128, nt, F], f16, tag=f"m{b}")
                    in_ap = table_full[0:hb, :] if b == 0 else table_full[hb:n, :]
                    cb = colbase[s]
                    ix = gixp.tile([128, CALL_TILES * 8], dt.int16, tag="gix")
                    nc.sync.dma_start(ix[:, :nt * 8], gidx[:, cb:cb + nt * 8])
                    nc.gpsimd.dma_gather(
                        mt[:], in_ap, ix[:, :nt * 8],
                        nt * TILE, nt * TILE, F,
                        single_packet=SP)
                    for j in range(nt):
                        msg_tiles[s + j] = (mt, j)
                    if SUB <= 1:
                        cs = sb.tile([128, F], f32, tag="y1")
                        nc.vector.tensor_copy(cs[:], mt[:, 0, :])
                        nc.sync.dma_start(dbg, cs[:])
                if SUB <= 1:
                    return
                for w_ in range(nw):
                    seq = ([toff[w_, 0] + i for i in range(T[w_, 0])]
                           + [toff[w_, 1] + i for i in range(T[w_, 1])])
                    ps_t = psW.tile([128, F], f32, tag="segacc")
                    if not seq:
                        nc.vector.memset(ps_t[:], 0.0)
                    for k, gt in enumerate(seq):
                        ind = indp.tile([128, 128], f16, tag="ind")
                        nc.vector.tensor_scalar(
                            ind[:], iota_sb[:], dslot_sb[:, gt:gt + 1], None,
                            op0=EQ)
                        mt, j = msg_tiles[gt]
                        if SUB <= 2:
                            continue
                        nc.tensor.matmul(ps_t[:], lhsT=ind[:], rhs=mt[:, j, :],
                                         start=(k == 0), stop=(k == len(seq) - 1))
                    if SUB <= 2:
                        cs = sb.tile([128, 128], f32, tag="y1")
                        nc.vector.tensor_copy(cs[:], ind[:])
                        nc.sync.dma_start(dbg, cs[:])
                        continue
                    if SUB <= 3:
                        cs = sb.tile([128, F], f32, tag="y1")
                        nc.vector.tensor_copy(cs[:], ps_t[:])
                        nc.sync.dma_start(dbg, cs[:])
                        continue
                    flush(w_, ps_t)

            # ---------- conv1 ----------
            h1T0 = bigp.tile([128, nw * 128], f16, tag="h1T0")
            h1T1 = bigp.tile([128, nw * 128], f16, tag="h1T1")

            def flush1(w_, ps_t):
                y1 = sb.tile([128, F], f32, tag="y1")
                nc.vector.scalar_tensor_tensor(
                    y1[:], in0=ps_t[:], scalar=dinv[:, w_:w_ + 1],
                    in1=u2_all[:, w_ * F:(w_ + 1) * F], op0=MUL, op1=ADD)
                pT = psT.tile([128, 128], f32, tag="pT")
                nc.tensor.transpose(pT[:], y1[:], ident_sb[:])
                aggT = sb.tile([128, 128], f32, tag="aggT")
                nc.vector.tensor_copy(aggT[:], pT[:])
                for o in range(2):
                    ph = psH.tile([128, 128], f32, tag="ph")
                    nc.tensor.matmul(ph[:], lhsT=W1_sb[:, o * 128:(o + 1) * 128],
                                     rhs=aggT[:], start=True, stop=True)
                    dstt = h1T0 if o == 0 else h1T1
                    nc.scalar.activation(dstt[:, w_ * 128:(w_ + 1) * 128], ph[:],
                                         RELU, bias=b1c_sb[:, o:o + 1], scale=1.0)

            if PH >= 1:
                segsum(u_full[:], flush1)

            # ---------- conv2 dense: w = dinv*(h1@W2) ----------
            w2_all = nbp.tile([128, nw * F], f32, tag="nodebuf")
            for w_ in range(nw if PH >= 2 else 0):
                pcnt = min(128, np_ - w_ * 128)
                pg = psH.tile([128, 128], f32, tag="ph")
                nc.tensor.matmul(pg[:], lhsT=W2a_sb[:],
                                 rhs=h1T0[:, w_ * 128:(w_ + 1) * 128],
                                 start=True, stop=False)
                nc.tensor.matmul(pg[:], lhsT=W2b_sb[:],
                                 rhs=h1T1[:, w_ * 128:(w_ + 1) * 128],
                                 start=False, stop=True)
                gT = sb.tile([128, 128], f32, tag="gT")
                nc.vector.tensor_copy(gT[:], pg[:])
                pN = psT.tile([128, 128], f32, tag="pT")
                nc.tensor.transpose(pN[:], gT[:], ident_sb[:])
                wt = sb.tile([128, F], f32, tag="wt")
                nc.vector.tensor_scalar_mul(wt[:], pN[:], dinv[:, w_:w_ + 1])
                wh = sb.tile([128, F], f16, tag="uh")
                nc.vector.tensor_copy(wh[:], wt[:])
                nc.sync.dma_start(w_blk[w_ * 128:w_ * 128 + pcnt, :], wh[:pcnt, :])
                nc.vector.tensor_scalar_mul(
                    w2_all[:, w_ * F:(w_ + 1) * F], wt[:], dinv[:, w_:w_ + 1])
            if PH >= 2:
                nc.gpsimd.collective_compute(
                    "AllGather", mybir.AluOpType.bypass, replica_groups=rg,
                    ins=[w_blk[:]], outs=[w_full[:]])


            # ---------- conv2 segsum + y + node head ----------
            def flush2(w_, ps_t):
                pcnt = min(128, np_ - w_ * 128)
                yt = sb.tile([128, F], f32, tag="y1")
                nc.vector.scalar_tensor_tensor(
                    yt[:], in0=ps_t[:], scalar=dinv[:, w_:w_ + 1],
                    in1=w2_all[:, w_ * F:(w_ + 1) * F], op0=MUL, op1=ADD)
                yh = sb.tile([128, F], f16, tag="uh")
                nc.vector.tensor_copy(yh[:], yt[:])
                nc.sync.dma_start(y_blk[w_ * 128:w_ * 128 + pcnt, :], yh[:pcnt, :])
                pT = psT.tile([128, 128], f32, tag="pT")
                nc.tensor.transpose(pT[:], yt[:], ident_sb[:])
                yT = sb.tile([128, 128], f32, tag="aggT")
                nc.vector.tensor_copy(yT[:], pT[:])
                pz = psZ.tile([C, 128], f32, tag="pz")
                nc.tensor.matmul(pz[:], lhsT=Qn_sb[:], rhs=yT[:],
                                 start=True, stop=True)
                nt_sb = sb.tile([C, 128], f32, tag="ntb")
                nc.scalar.activation(nt_sb[:], pz[:], ID,
                                     bias=cn_sb[:, 0:1], scale=1.0)
                nc.sync.dma_start(nodeT[:, w_ * 128:w_ * 128 + pcnt],
                                  nt_sb[:, :pcnt])

            if PH >= 3:
                segsum(w_full[:], flush2)
                nc.gpsimd.collective_compute(
                    "AllGather", mybir.AluOpType.bypass, replica_groups=rg,
                    ins=[y_blk[:]], outs=[y_full[:]])


            # ---------- edge phase ----------
            for (s, nt, b) in (calls if PH >= 4 else []):
                cb = colbase[s]
                nidx = nt * TILE
                ixs = gixp.tile([128, CALL_TILES * 8], dt.int16, tag="gix")
                nc.sync.dma_start(ixs[:, :nt * 8], gidx[:, cb:cb + nt * 8])
                ixd = gixp.tile([128, CALL_TILES * 8], dt.int16, tag="gixd")
                nc.sync.dma_start(ixd[:, :nt * 8], edst[:, cb:cb + nt * 8])
                ys = edp.tile([128, 1, nidx], f16, tag="ys")
                in_ap = y_full[0:hb, :] if b == 0 else y_full[hb:n, :]
                nc.gpsimd.dma_gather(ys[:], in_ap, ixs[:, :nt * 8],
                                     nidx, nidx, F, transpose=True,
                                     single_packet=False)
                yd = edp.tile([128, 1, nidx], f16, tag="yd")
                nc.gpsimd.dma_gather(yd[:], y_blk[:], ixd[:, :nt * 8],
                                     nidx, nidx, F, transpose=True,
                                     single_packet=False)
                ep_t = edp.tile([PP, nidx], f16, tag="ept")
                nc.sync.dma_start(ep_t[:], ept[:, s * TILE:s * TILE + nidx])
                for ch in range(0, nidx, 512):
                    ce_n = min(512, nidx - ch)
                    pse = psW.tile([C, 512], f32, tag="segacc")
                    nc.tensor.matmul(pse[:, :ce_n], lhsT=Qs16[:],
                                     rhs=ys[:, 0, ch:ch + ce_n],
                                     start=True, stop=False)
                    nc.tensor.matmul(pse[:, :ce_n], lhsT=Qd16[:],
                                     rhs=yd[:, 0, ch:ch + ce_n],
                                     start=False, stop=False)
                    nc.tensor.matmul(pse[:, :ce_n], lhsT=Wep_sb[:],
                                     rhs=ep_t[:, ch:ch + ce_n],
                                     start=False, stop=True)
                    eo = sb.tile([C, 512], f32, tag="eo")
                    nc.scalar.activation(eo[:, :ce_n], pse[:, :ce_n], ID,
                                         bias=ce_sb[:, 0:1], scale=1.0)
                    nc.sync.dma_start(
                        eoutT[:, s * TILE + ch:s * TILE + ch + ce_n],
                        eo[:, :ce_n])

    nc.compile()
    return nc


# --------------------------------------------------------------------------
# top-level entry
# --------------------------------------------------------------------------

def run_graph(inputs, n=N, m=M):
    x = np.ascontiguousarray(np.asarray(inputs["x"], np.float32))
    ei = np.asarray(inputs["edge_index"])
    ep = np.ascontiguousarray(np.asarray(inputs["edge_properties"], np.float32))
    W1 = np.asarray(inputs["W1"], np.float32)
    b1 = np.asarray(inputs["b1"], np.float32)
    W2 = np.asarray(inputs["W2"], np.float32)
    b2 = np.asarray(inputs["b2"], np.float32)
    Wd = np.asarray(inputs["Wd"], np.float32)
    bd = np.asarray(inputs["bd"], np.float32)
    Wn = np.asarray(inputs["Wn"], np.float32)
    bn = np.asarray(inputs["bn"], np.float32)
    We = np.asarray(inputs["We"], np.float32)
    be = np.asarray(inputs["be"], np.float32)
    E = ei.shape[1]

    src = ei[0].astype(np.int64)
    dst = ei[1].astype(np.int64)
    cores, S = prep(src, dst, n=n, m=m)
    nw, np_, TT, L = S["nw"], S["np_"], S["TT"], S["L"]

    f16n = np.float16
    iota_r = np.tile(np.arange(128, dtype=np.float32)[None, :], (128, 1)).astype(f16n)
    ident = np.eye(128, dtype=np.float32)
    common = dict(
        iota_r=iota_r, ident=ident,
        W1_i=W1, W2a_i=W2[0:128].astype(f16n), W2b_i=W2[128:256].astype(f16n),
        WdT_i=np.ascontiguousarray(Wd.T), Wn_i=Wn,
        Wes_i=np.ascontiguousarray(We[0:HD]),
        Wed_i=np.ascontiguousarray(We[HD:2 * HD]),
        Wep_i=We[128:131].astype(f16n),
        b1c_i=np.ascontiguousarray(b1.reshape(2, 128).T),
        b2c_i=b2.reshape(128, 1),
        bdc_i=bd.reshape(HD, 1),
        bnr_i=bn.reshape(1, C), ber_i=be.reshape(1, C),
        one_i=np.ones((1, 1), np.float32),
    )

    in_maps = []
    for c in range(m):
        core = cores[c]
        degp = np.ones((128, nw), np.float32)
        dv = core["deg"]
        for t in range(nw):
            pcnt = min(128, np_ - t * 128)
            degp[:pcnt, t] = dv[t * 128:t * 128 + pcnt]
        dslot = np.ascontiguousarray(
            core["dsl"].reshape(TT, TILE).T)           # [128, TT]
        slot = core["slot"]
        ept = np.ascontiguousarray(
            ep[np.where(slot >= 0, slot, 0)].T).astype(f16n)   # [3, L]
        in_maps.append(dict(
            common,
            x_blk=x[c * np_:(c + 1) * np_],
            deg_t=degp,
            gidx=wrap_idx(core["g16"], S),
            edst=wrap_idx(core["ed16"], S),
            dslot=dslot,
            ept=ept,
        ))

    nc = build_nc(S)
    trace = bool(os.environ.get("KBENCH_TRACE"))
    res = run_bass_kernel_spmd(nc, in_maps, core_ids=list(range(m)),
                               trace=trace)
    last_run_info["exec_time_ns"] = res.exec_time_ns
    last_run_info["mean_exec_time_ns"] = res.mean_exec_time_ns
    last_run_info["profile_json"] = res.profile_json

    node_x = np.zeros((n, C), np.float32)
    edge_x = np.zeros((E, C), np.float32)
    for c in range(m):
        r = res.results[c]
        node_x[c * np_:(c + 1) * np_] = r["nodeT"].T
        slot = cores[c]["slot"]
        valid = slot >= 0
        edge_x[slot[valid]] = r["eoutT"].T[valid]
    return node_x, edge_x


def kernel(**inputs):
    return run_graph(inputs, n=N, m=M)
